# revision 3
# baseline (speedup 1.0000x reference)
"""Trainium2 Bass kernel for nn_MultiHeadAttention (B=4, S=2048, D=1024, H=16).

Tensor-parallel over heads (2 heads/core, 8 cores). Key structure:
  * One continuous PE instruction stream: projection matmuls for batch b+1
    and output-projection work for earlier strips are interleaved as
    "pieces" between attention groups to cover the exp (ACT) latency
    (the PE p-state reaches 2.4 GHz only after ~3us of continuous work).
  * Scores matmuls (contraction d_k=64) use 64-row PE array tiling:
    head 0 on tile (0,0), head 1 on tile (64,0), executing concurrently.
  * V projection is computed seq-major directly (stationary = x chunk),
    eliminating PE transposes; the V bias is folded into the output bias
    on the host (softmax rows sum to 1 so it passes through attention).
  * Softmax normalization is deferred past the collective: the payload is
    unnormalized AV plus denominator rows, bf16.
  * Per-strip AllToAll: each core receives exactly the 64-column slice it
    out-projects. Staging buffers are row-major so the SBUF-side DMAs are
    contiguous; the collective reads/writes via strided DRAM views.
  * Output projection per strip-PAIR with column tiling: stationary is the
    normalized rhs (M=64); strips 2P and 2P+1 run on column tiles (0,0)
    and (0,64) concurrently while Wo streams 512 wide. The output bias is
    added on the host. rhs/den loads and output stores issue from the
    gpsimd software DGE so they never block the x/payload DMA queue.
  * All weight/x tensors are host-retiled so every big DMA is contiguous.
Host wraps: shards weights (1/sqrt(dk) folded into Wq/bq), classifies mask
blocks (drop / keep / unique [128, 512] tiles), reassembles + biases out.
"""

import ml_dtypes
import numpy as np

import concourse.bass as bass
import concourse.bacc as bacc
import concourse.mybir as mybir
import concourse.tile as tile
from concourse.bass_utils import run_bass_kernel_spmd

F32 = mybir.dt.float32
BF16 = mybir.dt.bfloat16
AF = mybir.ActivationFunctionType
OP = mybir.AluOpType

B, S, D_MODEL, N_HEADS, D_K = 4, 2048, 1024, 16, 64
N_CORES = 8
HPC = N_HEADS // N_CORES          # heads per core = 2
F = HPC * D_K                     # feature slice per core = 128
SEQ = B * S                       # 8192
S1B = 512                         # query-strip width
S2B = 128                         # key-chunk height
SP = S // S1B                     # 4 strips per batch
C2 = S // S2B                     # 16 key chunks per batch
KC = D_MODEL // 128               # 8 contraction chunks
SLC = S1B // N_CORES              # 64 out-proj seq cols per core per strip
PAY = F + HPC                     # 130 payload rows (AV + denominators)
A_DROP, A_KEEP = -2, -1

_nc_cache = {}


def _build_nc(actions_key, n_masks):
    actions = np.frombuffer(actions_key, dtype=np.int64).reshape(C2, SP)
    nc = bacc.Bacc("TRN2", target_bir_lowering=False, debug=False,
                   num_devices=N_CORES)

    xq = nc.dram_tensor("xq", [B, SP, 128, KC, S1B], BF16, kind="ExternalInput")
    xk = nc.dram_tensor("xk", [B, SP, 128, KC, S1B], BF16, kind="ExternalInput")
    xv = nc.dram_tensor("xv", [B, SP, 128, KC, S1B], BF16, kind="ExternalInput")
    wq = nc.dram_tensor("wq", [128, KC, F], BF16, kind="ExternalInput")
    wk = nc.dram_tensor("wk", [128, KC, F], BF16, kind="ExternalInput")
    wv = nc.dram_tensor("wv", [128, KC, F], BF16, kind="ExternalInput")
    bq = nc.dram_tensor("bq", [F, 1], F32, kind="ExternalInput")
    bk = nc.dram_tensor("bk", [F, 1], F32, kind="ExternalInput")
    wo = nc.dram_tensor("wo", [128, KC, KC, 128], BF16, kind="ExternalInput")
    sel8 = nc.dram_tensor("sel8", [128, KC, 128], BF16, kind="ExternalInput")
    masks = nc.dram_tensor("masks", [S2B, max(n_masks, 1), S1B], BF16,
                           kind="ExternalInput")

    agin2 = nc.dram_tensor("agin2", [B, 2, N_CORES, PAY, 2 * SLC], BF16)
    agf2 = nc.dram_tensor("agf2", [B, 2, N_CORES, PAY, 2 * SLC], BF16)
    out_t = nc.dram_tensor("out_t", [B, SP, SLC, D_MODEL], F32,
                           kind="ExternalOutput")

    with tile.TileContext(nc) as tc:
        with (
            tc.tile_pool(name="cst", bufs=1) as cst,
            tc.tile_pool(name="per", bufs=1) as per,
            tc.tile_pool(name="xin", bufs=13) as xin,
            tc.tile_pool(name="prp", bufs=3) as prp,
            tc.tile_pool(name="agp", bufs=2) as agp,
            tc.tile_pool(name="rhp", bufs=2) as rhp,
            tc.tile_pool(name="obp", bufs=2) as obp,
            tc.tile_pool(name="scp", bufs=2, space="PSUM") as scp,
            tc.tile_pool(name="avp", bufs=2, space="PSUM") as avp,
            tc.tile_pool(name="ppp", bufs=1, space="PSUM") as ppp,
            tc.tile_pool(name="opp", bufs=1, space="PSUM") as opp,
        ):
            wq_sb = cst.tile([128, KC, F], BF16, tag="wq")
            wk_sb = cst.tile([128, KC, F], BF16, tag="wk")
            wv_sb = cst.tile([128, KC, F], BF16, tag="wv")
            nc.sync.dma_start(wq_sb[:], wq[:])
            nc.sync.dma_start(wk_sb[:], wk[:])
            nc.sync.dma_start(wv_sb[:], wv[:])
            bq_sb = cst.tile([F, 1], F32, tag="bq")
            bk_sb = cst.tile([F, 1], F32, tag="bk")
            nc.sync.dma_start(bq_sb[:], bq[:])
            nc.sync.dma_start(bk_sb[:], bk[:])
            mk_sb = cst.tile([S2B, max(n_masks, 1), S1B], BF16, tag="mk")
            nc.scalar.dma_start(mk_sb[:], masks[:])
            wo_sb = cst.tile([128, KC, KC, 128], BF16, tag="wo")
            nc.scalar.dma_start(wo_sb[:], wo[:])
            sel_sb = cst.tile([128, KC, 128], BF16, tag="sel")
            nc.scalar.dma_start(sel_sb[:], sel8[:])
            rcp_sb = cst.tile([128, SLC], BF16, tag="rcp")
            nc.vector.memset(rcp_sb[:], 0.0)

            qT = [per.tile([F, S], BF16, tag=f"qT{i}", name=f"qT{i}")
                  for i in range(2)]
            kT = [per.tile([F, S], BF16, tag=f"kT{i}", name=f"kT{i}")
                  for i in range(2)]
            va = [per.tile([S2B, C2, HPC, D_K + 1], BF16, tag=f"va{i}",
                           name=f"va{i}") for i in range(2)]
            for t in va:
                nc.vector.memset(t[:, :, :, D_K:D_K + 1], 1.0)

            # ---------- piece machinery (PE-stream interleaving) ----------
            pend = []          # FIFO of (tag, key, closure)
            delayed = []       # (mature_tick, [(tag, key, closure), ...])
            tick = [0]

            def pump(n):
                for _ in range(min(n, len(pend))):
                    pend.pop(0)[2]()

            def force_proj(b, i1):
                # attention(b) strip i1 requires proj(b) strips <= i1 emitted
                while pend and pend[0][0] == "proj" and pend[0][1] <= (b, i1):
                    pend.pop(0)[2]()

            def mature():
                rest = []
                for mt, pieces in delayed:
                    if mt <= tick[0]:
                        pend.extend(pieces)
                    else:
                        rest.append((mt, pieces))
                delayed[:] = rest

            # ---------- projections ----------
            def emit_xdma(b, strips):
                tiles = {}
                for s in strips:
                    for nm, xd in (("q", xq), ("k", xk), ("v", xv)):
                        t = xin.tile([128, KC, S1B], BF16, tag="xt", name="xt")
                        nc.sync.dma_start(t[:], xd[b, s])
                        tiles[(nm, s)] = t
                return tiles

            def proj_pieces(b, xt, strips):
                pb = b % 2
                pcs = []

                def mk_qk(nm, dst, w_sb, b_sb, s):
                    box = {}
                    sl = slice(s * S1B, (s + 1) * S1B)

                    def p1():
                        ps = ppp.tile([128, S1B], F32, tag="pp", name="pp")
                        box["ps"] = ps
                        for kc in range(4):
                            nc.tensor.matmul(ps[:], w_sb[:, kc, :],
                                             xt[(nm, s)][:, kc, :],
                                             start=(kc == 0), stop=False)

                    def p2():
                        ps = box["ps"]
                        for kc in range(4, KC):
                            nc.tensor.matmul(ps[:], w_sb[:, kc, :],
                                             xt[(nm, s)][:, kc, :],
                                             start=False, stop=(kc == KC - 1))

                    def pe():
                        nc.vector.tensor_scalar_add(dst[:, sl], box["ps"][:],
                                                    b_sb[:, 0:1])

                    return [p1, p2, pe]

                def mk_v(s):
                    box = {}

                    def mk_chunk(j):
                        def pj():
                            if j == 0:
                                box["ps"] = ppp.tile([128, 4, S2B], F32,
                                                     tag="pp", name="pp")
                            vps = box["ps"]
                            for kc in range(KC):
                                nc.tensor.matmul(
                                    vps[:, j, :],
                                    xt[("v", s)][:, kc, j * S2B:(j + 1) * S2B],
                                    wv_sb[:, kc, :],
                                    start=(kc == 0), stop=(kc == KC - 1))
                        return pj

                    def pe():
                        nc.vector.tensor_copy(
                            va[pb][:, 4 * s:4 * s + 4, :, 0:D_K],
                            box["ps"][:].rearrange("p j (h d) -> p j h d",
                                                   h=HPC))

                    return [mk_chunk(j) for j in range(4)] + [pe]

                for s in strips:
                    for fn in mk_qk("k", kT[pb], wk_sb, bk_sb, s):
                        pcs.append(("proj", (b, s), fn))
                    for fn in mk_v(s):
                        pcs.append(("proj", (b, s), fn))
                    for fn in mk_qk("q", qT[pb], wq_sb, bq_sb, s):
                        pcs.append(("proj", (b, s), fn))
                return pcs

            # ---------- output projection (per batch, strip PAIR) ----------
            def outproj_pieces(b, pair):
                i1a, i1b = 2 * pair, 2 * pair + 1
                box = {}

                def p_load():
                    for t, i1 in enumerate((i1a, i1b)):
                        rhs = rhp.tile([128, KC, SLC], BF16, tag="rhs",
                                       name="rhs")
                        den = rhp.tile([16, SLC], BF16, tag="den", name="den")
                        nc.gpsimd.dma_start(
                            rhs[:],
                            agf2[b, pair, :, 0:F, t * SLC:(t + 1) * SLC]
                            .rearrange("g p c -> p g c"))
                        for lh in range(HPC):
                            nc.gpsimd.dma_start(
                                den[8 * lh:8 * lh + 8, :],
                                agf2[b, pair, :, F + lh,
                                     t * SLC:(t + 1) * SLC])
                        box[("rhs", t)], box[("den", t)] = rhs, den

                def mk_norm(t):
                    def p_norm():
                        with nc.allow_low_precision("bf16 recip of denom"):
                            nc.vector.reciprocal(rcp_sb[0:16, :],
                                                 box[("den", t)][0:16, :])
                        mtile = opp.tile([128, 4, 128], F32, tag="op",
                                         name="mult")
                        mflat = mtile.rearrange("p a b -> p (a b)")
                        for kc in range(KC):
                            nc.tensor.matmul(mflat[:, kc * SLC:(kc + 1) * SLC],
                                             sel_sb[:, kc, :], rcp_sb[:],
                                             start=True, stop=True)
                        rhn = rhp.tile([128, KC, SLC], BF16, tag="rhn",
                                       name="rhn")
                        nc.vector.tensor_tensor(
                            rhn[:].rearrange("p k c -> p (k c)"),
                            box[("rhs", t)][:].rearrange("p k c -> p (k c)"),
                            mflat[:], OP.mult)
                        box[("rhn", t)] = rhn
                    return p_norm

                def mk_op(half):
                    def p_op():
                        op = opp.tile([128, 4, 128], F32, tag="op", name="op")
                        box["op"] = op
                        for kc in range(KC):
                            for t in range(2):
                                nc.tensor.matmul(
                                    op[64 * t:64 * t + 64, :, :],
                                    box[("rhn", t)][:, kc, :],
                                    wo_sb[:, kc, 4 * half:4 * half + 4, :],
                                    start=(kc == 0), stop=(kc == KC - 1))
                    return p_op

                def mk_fin(half):
                    def p_fin():
                        ob = obp.tile([128, 4, 128], F32, tag="ob", name="ob")
                        nc.vector.tensor_copy(ob[:], box["op"][:])
                        for t, i1 in enumerate((i1a, i1b)):
                            nc.gpsimd.dma_start(
                                out_t[b, i1, :, 512 * half:512 * half + 512]
                                .rearrange("c (dc f) -> c dc f", f=128),
                                ob[64 * t:64 * t + 64, :, :])
                    return p_fin

                return [("op", (b, pair), fn) for fn in
                        (p_load, mk_norm(0), mk_norm(1),
                         mk_op(0), mk_fin(0), mk_op(1), mk_fin(1))]

            # ---------- attention ----------
            def emit_attention(b):
                pb = b % 2
                for i1 in range(SP):
                    force_proj(b, i1)
                    mature()
                    pump(3)
                    kept = [i2 for i2 in range(C2) if actions[i2, i1] != A_DROP]
                    groups = [kept[i:i + 2] for i in range(0, len(kept), 2)]
                    avs = [avp.tile([D_K + 1, S1B], F32, tag="av",
                                    name=f"av{lh}") for lh in range(HPC)]
                    q_strip = [qT[pb][lh * D_K:(lh + 1) * D_K,
                                      i1 * S1B:(i1 + 1) * S1B]
                               for lh in range(HPC)]

                    def emit_av(grp, prbs, nd, is_last):
                        for ci, i2 in enumerate(grp):
                            for lh in range(HPC):
                                nc.tensor.matmul(
                                    avs[lh][:], va[pb][:, i2, lh, :],
                                    prbs[lh][:, ci, :],
                                    start=(nd + ci == 0),
                                    stop=(is_last and ci == len(grp) - 1))

                    prev = None
                    n_done = 0
                    for grp in groups:
                        sc_pair = [scp.tile([S2B, 2, S1B], F32, tag="sc",
                                            name=f"sc{lh}")
                                   for lh in range(HPC)]
                        for ci, i2 in enumerate(grp):
                            for lh in range(HPC):
                                r0 = lh * D_K
                                nc.tensor.matmul(
                                    sc_pair[lh][:, ci, :],
                                    kT[pb][r0:r0 + D_K,
                                           i2 * S2B:(i2 + 1) * S2B],
                                    q_strip[lh], start=True, stop=True)
                        prbs = []
                        for lh in range(HPC):
                            pr = prp.tile([S2B, 2, S1B], BF16, tag="pr",
                                          name="pr")
                            nc.scalar.activation(pr[:, 0:len(grp), :],
                                                 sc_pair[lh][:, 0:len(grp), :],
                                                 AF.Exp)
                            for ci, i2 in enumerate(grp):
                                a = actions[i2, i1]
                                if a >= 0:
                                    nc.vector.tensor_tensor(
                                        pr[:, ci, :], pr[:, ci, :],
                                        mk_sb[:, a, :], OP.mult)
                            prbs.append(pr)
                        pump(3)
                        if prev is not None:
                            emit_av(*prev, is_last=False)
                        prev = (grp, prbs, n_done)
                        n_done += len(grp)
                    emit_av(*prev, is_last=True)

                    ag = agp.tile([S2B, HPC, S1B], BF16, tag="ag", name="ag")
                    for lh in range(HPC):
                        nc.vector.tensor_copy(ag[0:D_K + 1, lh, :], avs[lh][:])
                    half = i1 % 2
                    agin_r = agin2[b, i1 // 2].rearrange("d r c -> r d c")
                    for lh in range(HPC):
                        nc.sync.dma_start(
                            agin_r[lh * D_K:(lh + 1) * D_K, :,
                                   half * SLC:(half + 1) * SLC],
                            ag[0:D_K, lh, :].rearrange("p (d c) -> p d c",
                                                       d=N_CORES))
                        nc.sync.dma_start(
                            agin_r[F + lh:F + lh + 1, :,
                                   half * SLC:(half + 1) * SLC],
                            ag[D_K:D_K + 1, lh, :].rearrange(
                                "p (d c) -> p d c", d=N_CORES))
                    if half == 1:
                        nc.gpsimd.collective_compute(
                            "AllToAll", OP.bypass,
                            ins=[agin2[b, i1 // 2]], outs=[agf2[b, i1 // 2]],
                            replica_groups=[list(range(N_CORES))])
                        delayed.append((tick[0] + 1,
                                        outproj_pieces(b, i1 // 2)))
                    tick[0] += 1

            # ---------- main emission ----------
            xt0 = emit_xdma(0, range(SP))
            for _tag, _key, fn in proj_pieces(0, xt0, range(2)):
                fn()
            pend.extend(proj_pieces(0, xt0, range(2, SP)))
            for b in range(B):
                if b + 1 < B:
                    xt = emit_xdma(b + 1, range(SP))
                    pend.extend(proj_pieces(b + 1, xt, range(SP)))
                emit_attention(b)
            mature()
            while pend:
                pend.pop(0)[2]()
            for _mt, pieces in delayed:
                for _tag, _key, fn in pieces:
                    fn()
            delayed.clear()

    nc.finalize()
    return nc


def _classify_mask(mask):
    """Block-classify mask[0,0] on the scoresT grid: per (key-chunk i2,
    query-strip i1) -> drop / keep / index of a unique [128, 512] 0/1 tile."""
    m2 = np.asarray(mask)[0, 0] != 0  # [S, S], m2[q, k]
    actions = np.full((C2, SP), A_DROP, dtype=np.int64)
    uniq, tiles = {}, []
    for i2 in range(C2):
        for i1 in range(SP):
            blk = m2[i1 * S1B:(i1 + 1) * S1B, i2 * S2B:(i2 + 1) * S2B].T
            if blk.all():
                actions[i2, i1] = A_KEEP
            elif blk.any():
                key = blk.tobytes()
                if key not in uniq:
                    uniq[key] = len(tiles)
                    tiles.append(np.ascontiguousarray(blk).astype(
                        ml_dtypes.bfloat16))
                actions[i2, i1] = uniq[key]
    arr = (np.stack(tiles) if tiles
           else np.zeros((1, S2B, S1B), dtype=ml_dtypes.bfloat16))
    return actions, arr


def _prep(inputs):
    q = np.asarray(inputs["query"], dtype=np.float32).reshape(SEQ, D_MODEL)
    k = np.asarray(inputs["key"], dtype=np.float32).reshape(SEQ, D_MODEL)
    v = np.asarray(inputs["value"], dtype=np.float32).reshape(SEQ, D_MODEL)
    bf = ml_dtypes.bfloat16

    def tile_x(x):
        # [B, SP, 128, KC, S1B]: (b,s,p,kc,c) = x[b*S+s*S1B+c, kc*128+p]
        return np.ascontiguousarray(
            x.reshape(B, SP, S1B, KC, 128).transpose(0, 1, 4, 3, 2)
        ).astype(bf)

    Wq = np.asarray(inputs["Wq"], dtype=np.float32)
    Wk = np.asarray(inputs["Wk"], dtype=np.float32)
    Wv = np.asarray(inputs["Wv"], dtype=np.float32)
    Wo = np.asarray(inputs["Wo"], dtype=np.float32)
    bq = np.asarray(inputs["bq"], dtype=np.float32)
    bk = np.asarray(inputs["bk"], dtype=np.float32)
    bv = np.asarray(inputs["bv"], dtype=np.float32)
    bo = np.asarray(inputs["bo"], dtype=np.float32)

    scale = 1.0 / np.sqrt(D_K)
    actions, mask_tiles = _classify_mask(inputs["mask"])

    # exp-overflow guard for the no-max-subtract softmax
    qn = q @ Wq.T + bq
    kn = k @ Wk.T + bk
    qmax = np.linalg.norm(qn.reshape(SEQ, N_HEADS, D_K), axis=-1).max()
    kmax = np.linalg.norm(kn.reshape(SEQ, N_HEADS, D_K), axis=-1).max()
    assert scale * qmax * kmax < 80.0, \
        "score bound too large for exp without max-subtraction"

    def tile_w(w):  # [1024, fout] -> [128, KC, fout]
        return np.ascontiguousarray(
            w.reshape(KC, 128, w.shape[1]).transpose(1, 0, 2)).astype(bf)

    # rcp row for head h=2*kc+(p>=64) sits at partition (h%2)*8 + h//2
    sel8 = np.zeros((128, KC, 128), dtype=np.float32)
    for kc in range(KC):
        for p in range(128):
            sel8[kc + (8 if p >= 64 else 0), kc, p] = 1.0

    shared = {
        "xq": tile_x(q), "xk": tile_x(k), "xv": tile_x(v),
        "wo": np.ascontiguousarray(
            Wo.T.reshape(KC, 128, KC, 128).transpose(1, 0, 2, 3)).astype(bf),
        "sel8": sel8.astype(bf),
        "masks": np.ascontiguousarray(mask_tiles.transpose(1, 0, 2)),
    }
    in_maps = []
    for c in range(N_CORES):
        sl = slice(c * F, (c + 1) * F)
        m = dict(shared)
        m["wq"] = tile_w(np.ascontiguousarray((Wq[sl] * scale).T))
        m["wk"] = tile_w(np.ascontiguousarray(Wk[sl].T))
        m["wv"] = tile_w(np.ascontiguousarray(Wv[sl].T))
        m["bq"] = np.ascontiguousarray((bq[sl] * scale).reshape(F, 1))
        m["bk"] = np.ascontiguousarray(bk[sl].reshape(F, 1))
        in_maps.append(m)
    bo2 = (Wo @ bv + bo).astype(np.float32)
    return in_maps, actions, mask_tiles, bo2


def _run(inputs, trace=False, trace_cores=None):
    in_maps, actions, mask_tiles, bo2 = _prep(inputs)
    key = (actions.tobytes(), len(mask_tiles))
    if key not in _nc_cache:
        _nc_cache[key] = _build_nc(key[0], key[1])
    nc = _nc_cache[key]
    res = run_bass_kernel_spmd(nc, in_maps, list(range(N_CORES)),
                               trace=trace, trace_cores=trace_cores)
    out = np.empty((SEQ, D_MODEL), dtype=np.float32)
    for c in range(N_CORES):
        ot = res.results[c]["out_t"]  # [B, SP, SLC, D]
        for b in range(B):
            for i1 in range(SP):
                r = b * S + i1 * S1B + c * SLC
                out[r:r + SLC] = ot[b, i1]
    out += bo2[None, :]
    return out.reshape(B, S, D_MODEL), res


def kernel(**inputs) -> np.ndarray:
    out, _ = _run(inputs)
    return out
